# revision 15
# baseline (speedup 1.0000x reference)
"""Trainium2 Bass kernel for CARE position encoding (rotor sandwich).

out = R x R~ factorizes into 4 sequential Givens stages (blades 6,9,5,3).
This implementation:
  - computes all cos/sin tables on the HOST (from pos/theta/coefs) and
    ships them as fp16 -- the device does zero transcendental work;
  - stores x per-core in a position-innermost "slot" layout
    X[partition, slot*J + j] (J=256 positions per partition, 14 slots;
    multivector components 0 and 15 are invariant and bypass the device);
  - each Givens stage is 3 (or 6) DVE tensor_tensor ops in fp16, whose
    access patterns have unit-stride 256-long innermost runs -> the DVE
    runs them in 2x_1P packed mode (verified on HW);
  - the slot permutation was chosen so every stage's pair structure is an
    affine "grid" slot(q,e) = s0 + dq*q + de*e expressible in <=3 free
    AP dims (planes 6,3 as one op-triple; planes 9,5 as two halves).

Sign conventions (tau = Cayley sign of the rotated pair) are baked into
per-sub sign tables SS[r*J + j], r = q + nq*e, so arbitrary per-pair
orientations are free.
"""
import numpy as np

import concourse.bass as bass
import concourse.tile as tile
from concourse import bacc, mybir
from concourse.bass_utils import run_bass_kernel_spmd

F16 = mybir.dt.float16
F32 = mybir.dt.float32

P = 128
NCORES = 8
B, L, MV = 16, 16384, 16
MAX_LEN = 16384
ROWS_PER_CORE = B // NCORES          # 2
N = ROWS_PER_CORE * L                # 32768 positions per core
J = N // P                           # 256 positions per partition
NSLOT = 14

PLANE_BLADES = (3, 5, 9, 6)          # reference order (stage order reversed)
STAGE_ORDER = (6, 9, 5, 3)           # innermost rotor applied first

# slot[comp] for comps 1..14 (0 and 15 bypass the device entirely)
SLOT = {1: 3, 2: 13, 3: 9, 4: 6, 5: 2, 6: 12, 7: 5, 8: 8,
        9: 1, 10: 11, 11: 7, 12: 4, 13: 0, 14: 10}
COMPS = [c for c in range(MV) if c not in (0, 15)]
SLOT_TO_COMP = {s: c for c, s in SLOT.items()}

# Per-stage sub-ops: (nq, dq, de, s0, placement) with
# slot(comp placement[q][e]) = s0 + dq*q + de*e ; validated vs Cayley below.
STAGE_SUBS = {
    6: [(4, -2, 7, 6, ((4, 2), (12, 10), (5, 3), (13, 11)))],
    9: [(2, 2, 5, 3, ((1, 8), (7, 14))),
        (2, -7, -2, 11, ((10, 3), (12, 5)))],
    5: [(2, 6, 3, 3, ((1, 4), (3, 6))),
        (2, 6, 3, 1, ((9, 12), (11, 14)))],
    3: [(4, -1, 10, 3, ((1, 2), (5, 6), (9, 10), (13, 14)))],
}

# Table layout (units of J elements per partition), one CC + one shared SS
# per PLANE.  SS rows: m6/m9/m3 uniform-tau -> 2 rows [+s2, -s2]; m5 mixed
# pattern (+,-) per half -> 4 rows [s2, -s2, -s2, s2] (r = 2q + e).
# Stage order: m6 (3J) | m9 (3J) | m5 (5J) | m3 (3J) = 14J total.
_TBL_PLANE = {6: (0, 1, 2), 9: (3, 4, 2), 5: (6, 7, 4), 3: (11, 12, 2)}
TBL_J = 14

# slots 4..9 are final after stage 5 (not touched by stage 3)
EARLY_OUT = (4, 10)                   # slot range [4, 10)
LATE_OUT = ((0, 4), (10, 14))


def _build_cayley(k=4):
    n = 1 << k
    C = np.zeros((n, n, n), dtype=np.float32)
    for a in range(n):
        for b in range(n):
            s, t = 0, a >> 1
            while t:
                s += bin(t & b).count("1")
                t >>= 1
            C[a, b, a ^ b] = -1.0 if (s & 1) else 1.0
    return C


def _verify_layout(cayley):
    """Check SLOT/STAGE_SUBS against the runtime Cayley tensor."""
    for m in STAGE_ORDER:
        rotated = set()
        for (nq, dq, de, s0, placement) in STAGE_SUBS[m]:
            for q, (a, b) in enumerate(placement):
                assert b == (a ^ m), (m, a, b)
                assert SLOT[a] == s0 + dq * q, (m, q, a)
                assert SLOT[b] == s0 + dq * q + de, (m, q, b)
                assert abs(cayley[a, m, b]) == 1.0
                rotated |= {a, b}
        expect = {c for c in range(MV) if bin(c & m).count("1") % 2 == 1}
        assert rotated == expect, (m, rotated, expect)


def _ap(base_ap, extra_off, dims):
    ap = [list(base_ap.ap[0])] + [list(d) for d in dims]
    return bass.AP(base_ap.tensor, base_ap.offset + extra_off, ap)


def _build_program():
    nc = bacc.Bacc("TRN2", target_bir_lowering=False, debug=False,
                   enable_asserts=False, num_devices=NCORES)
    x_d = nc.dram_tensor("x", [P, NSLOT * J], F16, kind="ExternalInput")
    t_d = nc.dram_tensor("tbl", [P, TBL_J * J], F16, kind="ExternalInput")
    out_d = nc.dram_tensor("out", [P, NSLOT * J], F16, kind="ExternalOutput")

    cayley = _build_cayley()

    def ss_ap(TBL, m, sub):
        nq, dq, de, s0, placement = sub
        ss_j = _TBL_PLANE[m][1]
        tau0 = [float(cayley[a, m, b]) for (a, b) in placement]
        if all(t == tau0[0] for t in tau0):
            t = tau0[0]
            off = ss_j * J + (0 if t > 0 else J)
            estep = J if t > 0 else -J
            return _ap(TBL[:], off, [[0, nq], [estep, 2], [1, J]])
        assert nq == 2 and tau0 == [1.0, -1.0], (m, tau0)
        return _ap(TBL[:], ss_j * J, [[2 * J, nq], [J, 2], [1, J]])

    with tile.TileContext(nc) as tc:
        with tc.tile_pool(name="data", bufs=1) as dpool, \
             tc.tile_pool(name="tu", bufs=1) as tupool:
            TBL = dpool.tile([P, TBL_J * J], F16)
            X = dpool.tile([P, NSLOT * J], F16)

            # spread input DMAs across idle engine queues so issues (and,
            # if the rings allow, transfers) run in parallel after the
            # framework preamble barrier
            nc.sync.dma_start(TBL[:, :3 * J], t_d[:, :3 * J])
            nc.sync.dma_start(X[:], x_d[:])
            nc.sync.dma_start(TBL[:, 3 * J:], t_d[:, 3 * J:])

            # Split planes 9/5: T and A don't care about pair structure, so
            # they run as ONE merged FD=8J op over the union of both halves'
            # cells; only the U ops stay per-half.  (Validated vs reference.)
            MERGED = {
                9: dict(x_off=2 * J,
                        x_dims=[[6 * J, 2], [J, 4], [1, J]],
                        tu_dims=[[4 * J, 2], [J, 4], [1, J]],
                        cc_dims=[[0, 2], [0, 4], [1, J]],
                        usubs=[
                            dict(u_off=J,
                                 u_dims=[[2 * J, 2], [3 * J, 2], [1, J]],
                                 xp_off=8 * J,
                                 xp_dims=[[2 * J, 2], [-5 * J, 2], [1, J]],
                                 ss_off=0,
                                 ss_dims=[[0, 2], [J, 2], [1, J]]),
                            dict(u_off=7 * J,
                                 u_dims=[[-5 * J, 2], [-2 * J, 2], [1, J]],
                                 xp_off=9 * J,
                                 xp_dims=[[-7 * J, 2], [2 * J, 2], [1, J]],
                                 ss_off=0,
                                 ss_dims=[[0, 2], [J, 2], [1, J]]),
                        ]),
                5: dict(x_off=J,
                        x_dims=[[3 * J, 4], [2 * J, 2], [1, J]],
                        tu_dims=[[2 * J, 4], [J, 2], [1, J]],
                        cc_dims=[[0, 4], [0, 2], [1, J]],
                        usubs=[
                            dict(u_off=J,
                                 u_dims=[[4 * J, 2], [2 * J, 2], [1, J]],
                                 xp_off=6 * J,
                                 xp_dims=[[6 * J, 2], [-3 * J, 2], [1, J]],
                                 ss_off=0,
                                 ss_dims=[[2 * J, 2], [J, 2], [1, J]]),
                            dict(u_off=0,
                                 u_dims=[[4 * J, 2], [2 * J, 2], [1, J]],
                                 xp_off=4 * J,
                                 xp_dims=[[6 * J, 2], [-3 * J, 2], [1, J]],
                                 ss_off=0,
                                 ss_dims=[[2 * J, 2], [J, 2], [1, J]]),
                        ]),
            }

            for m in STAGE_ORDER:
                cc_j = _TBL_PLANE[m][0]
                if m in MERGED:
                    sp = MERGED[m]
                    ss_j = _TBL_PLANE[m][1]
                    T = tupool.tile([P, 8 * J], F16, tag="t")
                    U = tupool.tile([P, 8 * J], F16, tag="u")
                    nc.vector.tensor_mul(
                        _ap(T[:], 0, sp["tu_dims"]),
                        _ap(X[:], sp["x_off"], sp["x_dims"]),
                        _ap(TBL[:], cc_j * J, sp["cc_dims"]))
                    for us in sp["usubs"]:
                        nc.vector.tensor_mul(
                            _ap(U[:], us["u_off"], us["u_dims"]),
                            _ap(X[:], us["xp_off"], us["xp_dims"]),
                            _ap(TBL[:], ss_j * J + us["ss_off"],
                                us["ss_dims"]))
                    nc.vector.tensor_add(
                        _ap(X[:], sp["x_off"], sp["x_dims"]),
                        _ap(T[:], 0, sp["tu_dims"]),
                        _ap(U[:], 0, sp["tu_dims"]))
                    if m == 5:
                        a, b = EARLY_OUT
                        nc.sync.dma_start(out_d[:, a * J:b * J],
                                          X[:, a * J:b * J])
                    continue
                for si, sub in enumerate(STAGE_SUBS[m]):
                    nq, dq, de, s0, placement = sub
                    fd = nq * 2 * J
                    T = tupool.tile([P, fd], F16, tag="t")
                    U = tupool.tile([P, fd], F16, tag="u")
                    grid = [[dq * J, nq], [de * J, 2], [1, J]]
                    tu_out = [[2 * J, nq], [J, 2], [1, J]]
                    # T = X[grid] * c2
                    nc.vector.tensor_mul(
                        _ap(T[:], 0, tu_out),
                        _ap(X[:], s0 * J, grid),
                        _ap(TBL[:], cc_j * J, [[0, nq], [0, 2], [1, J]]))
                    # U = X[partner] * (tau-signed s2)
                    nc.vector.tensor_mul(
                        _ap(U[:], 0, tu_out),
                        _ap(X[:], (s0 + de) * J,
                            [[dq * J, nq], [-de * J, 2], [1, J]]),
                        ss_ap(TBL, m, sub))
                    # X[grid] = T + U ; last stage: split by e-halves so the
                    # first output DMA overlaps the second add
                    if m == STAGE_ORDER[-1]:
                        half = [[dq * J, nq], [1, J]]
                        tu_half = [[2 * J, nq], [1, J]]
                        nc.vector.tensor_add(
                            _ap(X[:], s0 * J, half),
                            _ap(T[:], 0, tu_half), _ap(U[:], 0, tu_half))
                        nc.sync.dma_start(out_d[:, 0:4 * J], X[:, 0:4 * J])
                        nc.vector.tensor_add(
                            _ap(X[:], (s0 + de) * J, half),
                            _ap(T[:], J, tu_half), _ap(U[:], J, tu_half))
                        nc.sync.dma_start(out_d[:, 10 * J:14 * J],
                                          X[:, 10 * J:14 * J])
                    else:
                        nc.vector.tensor_add(
                            _ap(X[:], s0 * J, grid),
                            _ap(T[:], 0, tu_out),
                            _ap(U[:], 0, tu_out))
                if m == 5:
                    a, b = EARLY_OUT
                    nc.sync.dma_start(out_d[:, a * J:b * J],
                                      X[:, a * J:b * J])

    nc.compile()
    return nc


_PROGRAM_CACHE = {}


def _get_program():
    if "p" not in _PROGRAM_CACHE:
        _PROGRAM_CACHE["p"] = _build_program()
    return _PROGRAM_CACHE["p"]


def _build_in_maps(x, pos, coefs, theta0, cayley):
    """Host-side: slot-permuted fp16 x + per-core sign tables."""
    _verify_layout(cayley)
    # full-length cos/sin tables per plane: angle = theta0[p, i] * coef_i
    ang = theta0.astype(np.float64) * np.asarray(coefs, np.float64)[None, :]
    ctab = np.cos(ang).astype(np.float16)          # (MAX_LEN, 4)
    stab = np.sin(ang).astype(np.float16)
    plane_idx = {m: PLANE_BLADES.index(m) for m in STAGE_ORDER}

    pos_i = np.clip(pos, 0, MAX_LEN - 1).astype(np.int64)
    comp_order = [SLOT_TO_COMP[s] for s in range(NSLOT)]

    in_maps = []
    for g in range(NCORES):
        xr = np.ascontiguousarray(
            x[g * ROWS_PER_CORE:(g + 1) * ROWS_PER_CORE]
        ).reshape(P, J, MV)
        xs = xr.transpose(0, 2, 1)[:, comp_order, :]       # (P, 14, J)
        x16 = np.ascontiguousarray(xs).astype(np.float16).reshape(P, NSLOT * J)

        pg = pos_i[g * ROWS_PER_CORE:(g + 1) * ROWS_PER_CORE].reshape(P, J)
        tbl = np.empty((P, TBL_J, J), dtype=np.float16)
        for m in STAGE_ORDER:
            cc, ss, nrows = _TBL_PLANE[m]
            i = plane_idx[m]
            c2 = ctab[pg, i]                               # (P, J)
            s2 = stab[pg, i]
            tbl[:, cc, :] = c2
            if nrows == 2:
                tbl[:, ss, :] = s2
                tbl[:, ss + 1, :] = -s2
            else:                      # m5 pattern (+,-): rows s,-s,-s,s
                tbl[:, ss, :] = s2
                tbl[:, ss + 1, :] = -s2
                tbl[:, ss + 2, :] = -s2
                tbl[:, ss + 3, :] = s2
        in_maps.append({"x": x16, "tbl": tbl.reshape(P, TBL_J * J)})
    return in_maps


def kernel(x, pos, bx, by, bz, bw, theta, cayley, biv_mask, scalar_mask):
    x = np.asarray(x, dtype=np.float32)
    pos = np.asarray(pos)
    theta = np.asarray(theta, dtype=np.float32)
    cayley = np.asarray(cayley, dtype=np.float32)
    assert x.shape == (B, L, MV) and pos.shape == (B, L)

    coefs = [float(np.asarray(c, dtype=np.float32).reshape(MV)[b])
             for c, b in zip((bx, by, bz, bw), PLANE_BLADES)]
    theta0 = theta.reshape(MAX_LEN, 4)

    nc = _get_program()
    in_maps = _build_in_maps(x, pos, coefs, theta0, cayley)
    res = run_bass_kernel_spmd(nc, in_maps, core_ids=list(range(NCORES)))

    out = np.empty((B, L, MV), dtype=np.float32)
    comp_order = [SLOT_TO_COMP[s] for s in range(NSLOT)]
    for g in range(NCORES):
        r = res.results[g]["out"].reshape(P, NSLOT, J).astype(np.float32)
        og = np.empty((P, MV, J), dtype=np.float32)
        og[:, comp_order, :] = r
        xr = np.ascontiguousarray(
            x[g * ROWS_PER_CORE:(g + 1) * ROWS_PER_CORE]).reshape(P, J, MV)
        og[:, 0, :] = xr[:, :, 0]
        og[:, 15, :] = xr[:, :, 15]
        out[g * ROWS_PER_CORE:(g + 1) * ROWS_PER_CORE] = \
            og.transpose(0, 2, 1).reshape(ROWS_PER_CORE, L, MV)
    return out


# revision 16
# speedup vs baseline: 1.0424x; 1.0424x over previous
"""Trainium2 Bass kernel for CARE position encoding (rotor sandwich).

out = R x R~ factorizes into 4 sequential Givens stages (blades 6,9,5,3).
Implementation highlights:
  - all cos/sin tables computed on the HOST and shipped fp16; the device
    does no transcendental work and never sees `pos`;
  - x stored per-core position-innermost: X[partition, slot*J + j]
    (J=256, 14 slots; multivector comps 0/15 are invariant -> host copy);
  - every stage = 4 fp16 DVE tensor_tensor ops (merged T multiply, two
    half U multiplies, merged add) whose innermost dims are 256-long
    unit-stride runs -> DVE 2x_1P packed mode;
  - slot permutation chosen so each plane's 8 rotated cells form a 2-dim
    slot lattice {c0+a*i+d*k} (one full-width T/A op) and stage 6's cells
    are exactly slots 0..7, so the x DMA splits into an early gating
    chunk and the rest overlaps stage-6 compute;
  - stage-3 (last) add is split by lattice k-halves so output DMAs start
    while the second half computes.
"""
import numpy as np

import concourse.bass as bass
import concourse.tile as tile
from concourse import bacc, mybir
from concourse.bass_utils import run_bass_kernel_spmd

F16 = mybir.dt.float16
F32 = mybir.dt.float32

P = 128
NCORES = 8
B, L, MV = 16, 16384, 16
MAX_LEN = 16384
ROWS_PER_CORE = B // NCORES          # 2
N = ROWS_PER_CORE * L                # 32768 positions per core
J = N // P                           # 256 positions per partition
NSLOT = 14

PLANE_BLADES = (3, 5, 9, 6)          # reference arg order
STAGE_ORDER = (6, 9, 5, 3)           # innermost rotor applied first

# slot[comp]; comps 0 and 15 bypass the device (host passthrough)
SLOT = {1: 12, 2: 2, 3: 4, 4: 6, 5: 0, 6: 10, 7: 8, 8: 9,
        9: 11, 10: 1, 11: 7, 12: 5, 13: 3, 14: 13}
COMPS = [c for c in range(MV) if c not in (0, 15)]
SLOT_TO_COMP = {s: c for c, s in SLOT.items()}

# Per-plane merged spec: lat = (c0, a, d): cells {c0+a*i+d*k, i<4, k<2},
# T-tile col = 2i+k.  usubs: U-op (q,e) iteration: out col, partner-read
# slot, tau(e=0) per q.  All offsets/steps in SLOT units (scaled by J).
PLANE_SPECS = {
    6: dict(lat=(0, 1, 4), usubs=[
        dict(u_off=4, u_dims=[[-4, 2], [1, 2]],
             xp_off=6, xp_dims=[[-2, 2], [-4, 2]], tau=(1, -1)),
        dict(u_off=2, u_dims=[[4, 2], [1, 2]],
             xp_off=5, xp_dims=[[2, 2], [-4, 2]], tau=(1, -1))]),
    9: dict(lat=(0, 4, 1), usubs=[
        dict(u_off=6, u_dims=[[-4, 2], [-1, 2]],
             xp_off=9, xp_dims=[[-8, 2], [3, 2]], tau=(1, -1)),
        dict(u_off=3, u_dims=[[4, 2], [-3, 2]],
             xp_off=0, xp_dims=[[8, 2], [5, 2]], tau=(1, -1))]),
    5: dict(lat=(4, 1, 6), usubs=[
        dict(u_off=5, u_dims=[[-4, 2], [-1, 2]],
             xp_off=6, xp_dims=[[-2, 2], [6, 2]], tau=(1, 1)),
        dict(u_off=3, u_dims=[[4, 2], [-1, 2]],
             xp_off=5, xp_dims=[[2, 2], [6, 2]], tau=(1, 1))]),
    3: dict(lat=(0, 1, 10), usubs=[
        dict(u_off=5, u_dims=[[-4, 2], [-1, 2]],
             xp_off=2, xp_dims=[[-2, 2], [10, 2]], tau=(1, -1)),
        dict(u_off=3, u_dims=[[4, 2], [-1, 2]],
             xp_off=1, xp_dims=[[2, 2], [10, 2]], tau=(1, -1))]),
}

# tables: per plane CC [J] + SS 4 rows [s,-s,-s,s]; stage order
_TBL_CC = {6: 0, 9: 5, 5: 10, 3: 15}
TBL_J = 20

EARLY_OUT = (4, 10)                  # slots 4..9 final after stage 5


def _build_cayley(k=4):
    n = 1 << k
    C = np.zeros((n, n, n), dtype=np.float32)
    for a in range(n):
        for b in range(n):
            s, t = 0, a >> 1
            while t:
                s += bin(t & b).count("1")
                t >>= 1
            C[a, b, a ^ b] = -1.0 if (s & 1) else 1.0
    return C


def _verify_layout(cayley):
    """Re-derive every stage from SLOT/PLANE_SPECS and check against the
    runtime Cayley tensor via a tiny numeric simulation."""
    rng = np.random.default_rng(3)
    Jt = 8
    x = rng.standard_normal((MV, Jt))
    ang = rng.standard_normal((4, Jt))
    ref = x.copy()
    for si, m in enumerate(STAGE_ORDER):
        c2, s2 = np.cos(ang[si]), np.sin(ang[si])
        new = ref.copy()
        for a in range(MV):
            if bin(a & m).count("1") % 2 == 1:
                b = a ^ m
                new[a] = c2 * ref[a] + cayley[a, m, b] * s2 * ref[b]
        ref = new
    X = np.zeros((NSLOT, Jt))
    for c in COMPS:
        X[SLOT[c]] = x[c]
    for si, m in enumerate(STAGE_ORDER):
        c2, s2 = np.cos(ang[si]), np.sin(ang[si])
        sp = PLANE_SPECS[m]
        c0, a, d = sp["lat"]
        T = np.zeros((8, Jt))
        U = np.zeros((8, Jt))
        for i in range(4):
            for k in range(2):
                T[2 * i + k] = X[c0 + a * i + d * k] * c2
        for us in sp["usubs"]:
            for q in range(2):
                for e in range(2):
                    ucol = us["u_off"] + us["u_dims"][0][0] * q + \
                        us["u_dims"][1][0] * e
                    xs = us["xp_off"] + us["xp_dims"][0][0] * q + \
                        us["xp_dims"][1][0] * e
                    sgn = us["tau"][q] * (1.0 if e == 0 else -1.0)
                    U[ucol] = X[xs] * sgn * s2
        for i in range(4):
            for k in range(2):
                X[c0 + a * i + d * k] = T[2 * i + k] + U[2 * i + k]
    got = np.zeros((MV, Jt))
    got[0], got[15] = x[0], x[15]
    for c in COMPS:
        got[c] = X[SLOT[c]]
    assert np.abs(got - ref).max() < 1e-9, "layout/spec validation failed"


def _ap(base_ap, extra_off, dims):
    ap = [list(base_ap.ap[0])] + [list(d) for d in dims]
    return bass.AP(base_ap.tensor, base_ap.offset + extra_off, ap)


def _ss_dims(tau):
    """AP (offset_J, dims) into 4-row table [s,-s,-s,s] giving
    tau[q]*(-1)^e across (q,e)."""
    if tau == (1, 1):
        return 0, [[0, 2], [1, 2]]
    if tau == (-1, -1):
        return 1, [[0, 2], [-1, 2]]
    if tau == (1, -1):
        return 0, [[2, 2], [1, 2]]
    # (-1, 1): r = 1 - q + 2e
    return 1, [[-1, 2], [2, 2]]


def _build_program():
    nc = bacc.Bacc("TRN2", target_bir_lowering=False, debug=False,
                   enable_asserts=False, num_devices=NCORES)
    x_d = nc.dram_tensor("x", [P, NSLOT * J], F16, kind="ExternalInput")
    t_d = nc.dram_tensor("tbl", [P, TBL_J * J], F16, kind="ExternalInput")
    out_d = nc.dram_tensor("out", [P, NSLOT * J], F16, kind="ExternalOutput")

    with tile.TileContext(nc) as tc:
        with tc.tile_pool(name="data", bufs=1) as dpool, \
             tc.tile_pool(name="tu", bufs=1) as tupool:
            TBL = dpool.tile([P, TBL_J * J], F16)
            X = dpool.tile([P, NSLOT * J], F16)

            # stage-6 gate = its tables + x slots 0..7 only; the rest of x
            # and the later tables stream in under stage-6 compute
            nc.sync.dma_start(TBL[:, :5 * J], t_d[:, :5 * J])
            nc.sync.dma_start(X[:, :8 * J], x_d[:, :8 * J])
            nc.sync.dma_start(X[:, 8 * J:], x_d[:, 8 * J:])
            nc.sync.dma_start(TBL[:, 5 * J:], t_d[:, 5 * J:])

            tu_dims = [[2 * J, 4], [J, 2], [1, J]]
            for m in STAGE_ORDER:
                sp = PLANE_SPECS[m]
                c0, a, d = sp["lat"]
                cc_j = _TBL_CC[m]
                grid = [[a * J, 4], [d * J, 2], [1, J]]
                T = tupool.tile([P, 8 * J], F16, tag="t")
                U = tupool.tile([P, 8 * J], F16, tag="u")
                nc.vector.tensor_mul(
                    _ap(T[:], 0, tu_dims),
                    _ap(X[:], c0 * J, grid),
                    _ap(TBL[:], cc_j * J, [[0, 4], [0, 2], [1, J]]))
                for us in sp["usubs"]:
                    so, sd = _ss_dims(tuple(us["tau"]))
                    nc.vector.tensor_mul(
                        _ap(U[:], us["u_off"] * J,
                            [[us["u_dims"][0][0] * J, 2],
                             [us["u_dims"][1][0] * J, 2], [1, J]]),
                        _ap(X[:], us["xp_off"] * J,
                            [[us["xp_dims"][0][0] * J, 2],
                             [us["xp_dims"][1][0] * J, 2], [1, J]]),
                        _ap(TBL[:], (cc_j + 1 + so) * J,
                            [[sd[0][0] * J, 2], [sd[1][0] * J, 2], [1, J]]))
                if m == STAGE_ORDER[-1]:
                    # split add by k-halves; overlap output DMAs
                    for k in range(2):
                        nc.vector.tensor_add(
                            _ap(X[:], (c0 + d * k) * J,
                                [[a * J, 4], [1, J]]),
                            _ap(T[:], k * J, [[2 * J, 4], [1, J]]),
                            _ap(U[:], k * J, [[2 * J, 4], [1, J]]))
                        lo = c0 + d * k
                        nc.sync.dma_start(
                            out_d[:, lo * J:(lo + 4) * J],
                            X[:, lo * J:(lo + 4) * J])
                else:
                    nc.vector.tensor_add(
                        _ap(X[:], c0 * J, grid),
                        _ap(T[:], 0, tu_dims),
                        _ap(U[:], 0, tu_dims))
                if m == 5:
                    aa, bb = EARLY_OUT
                    nc.sync.dma_start(out_d[:, aa * J:bb * J],
                                      X[:, aa * J:bb * J])

    nc.compile()
    return nc


_PROGRAM_CACHE = {}


def _get_program():
    if "p" not in _PROGRAM_CACHE:
        _PROGRAM_CACHE["p"] = _build_program()
    return _PROGRAM_CACHE["p"]


def _build_in_maps(x, pos, coefs, theta0, cayley):
    """Host-side: slot-permuted fp16 x + per-core cos/sin tables."""
    _verify_layout(cayley)
    ang = theta0.astype(np.float64) * np.asarray(coefs, np.float64)[None, :]
    ctab = np.cos(ang).astype(np.float16)          # (MAX_LEN, 4)
    stab = np.sin(ang).astype(np.float16)
    plane_idx = {m: PLANE_BLADES.index(m) for m in STAGE_ORDER}

    pos_i = np.clip(pos, 0, MAX_LEN - 1).astype(np.int64)
    comp_order = [SLOT_TO_COMP[s] for s in range(NSLOT)]

    in_maps = []
    for g in range(NCORES):
        xr = np.ascontiguousarray(
            x[g * ROWS_PER_CORE:(g + 1) * ROWS_PER_CORE]
        ).reshape(P, J, MV)
        xs = xr.transpose(0, 2, 1)[:, comp_order, :]       # (P, 14, J)
        x16 = np.ascontiguousarray(xs).astype(np.float16).reshape(
            P, NSLOT * J)

        pg = pos_i[g * ROWS_PER_CORE:(g + 1) * ROWS_PER_CORE].reshape(P, J)
        tbl = np.empty((P, TBL_J, J), dtype=np.float16)
        for m in STAGE_ORDER:
            cc = _TBL_CC[m]
            i = plane_idx[m]
            c2 = ctab[pg, i]
            s2 = stab[pg, i]
            tbl[:, cc, :] = c2
            tbl[:, cc + 1, :] = s2
            tbl[:, cc + 2, :] = -s2
            tbl[:, cc + 3, :] = -s2
            tbl[:, cc + 4, :] = s2
        in_maps.append({"x": x16, "tbl": tbl.reshape(P, TBL_J * J)})
    return in_maps


def kernel(x, pos, bx, by, bz, bw, theta, cayley, biv_mask, scalar_mask):
    x = np.asarray(x, dtype=np.float32)
    pos = np.asarray(pos)
    theta = np.asarray(theta, dtype=np.float32)
    cayley = np.asarray(cayley, dtype=np.float32)
    assert x.shape == (B, L, MV) and pos.shape == (B, L)

    coefs = [float(np.asarray(c, dtype=np.float32).reshape(MV)[b])
             for c, b in zip((bx, by, bz, bw), PLANE_BLADES)]
    theta0 = theta.reshape(MAX_LEN, 4)

    nc = _get_program()
    in_maps = _build_in_maps(x, pos, coefs, theta0, cayley)
    res = run_bass_kernel_spmd(nc, in_maps, core_ids=list(range(NCORES)))

    out = np.empty((B, L, MV), dtype=np.float32)
    comp_order = [SLOT_TO_COMP[s] for s in range(NSLOT)]
    for g in range(NCORES):
        r = res.results[g]["out"].reshape(P, NSLOT, J).astype(np.float32)
        og = np.empty((P, MV, J), dtype=np.float32)
        og[:, comp_order, :] = r
        xr = np.ascontiguousarray(
            x[g * ROWS_PER_CORE:(g + 1) * ROWS_PER_CORE]).reshape(P, J, MV)
        og[:, 0, :] = xr[:, :, 0]
        og[:, 15, :] = xr[:, :, 15]
        out[g * ROWS_PER_CORE:(g + 1) * ROWS_PER_CORE] = \
            og.transpose(0, 2, 1).reshape(ROWS_PER_CORE, L, MV)
    return out
